# revision 6
# baseline (speedup 1.0000x reference)
"""Distributed Bass/Tile kernel for a dense transformer block on 8 TRN2 NeuronCores.

Sharding: sequence-parallel. Flattened tokens [B*S] are split into 8 chunks of
TOK=512 tokens; cores 0-3 hold batch 0, cores 4-7 batch 1. Each core computes
LN1 -> QKV for its chunk, AllGathers K^T and V (groups of 4 = one batch),
runs full attention for its query chunk, then proj+residual, LN2, and the FFN
row-parallel with replicated weights. No all-reduce is needed.

On-chip layout is feature-major ([feature, token]); all host<->device
transposes happen in numpy inside kernel().

Matmuls run in bf16 (fp32 PSUM accumulation); residual stream stays fp32.
Softmax skips max-subtraction (scores*scale is bounded ~±4 for this model)
and folds the denominator into the AV matmul via a ones-augmented V.
"""

import numpy as np
import ml_dtypes

import concourse.bacc as bacc
import concourse.mybir as mybir
import concourse.tile as tile
from concourse.bass_utils import run_bass_kernel_spmd

F32 = mybir.dt.float32
BF16 = mybir.dt.bfloat16

FULL_DIMS = dict(E=1024, H=16, DH=64, TOK=512, G=4, NC=8, FF=4096)


def build_nc(dims):
    E, H, DH, TOK, G, NC, FF = (
        dims["E"], dims["H"], dims["DH"], dims["TOK"], dims["G"], dims["NC"], dims["FF"]
    )
    ET = E // 128          # embedding 128-tiles
    FT = FF // 128         # ffn-hidden 128-tiles
    TOKT = TOK // 128      # token 128-tiles per core
    KT = G * TOKT          # total key tiles per batch group
    HD1 = DH + 1           # V columns + ones column
    NV = (H * DH + 511) // 512   # 512-wide column chunks of V
    HPN = 512 // DH              # heads per V column chunk
    eps = 1e-5
    sm_scale = float(DH) ** -0.5
    add, mult, mx = mybir.AluOpType.add, mybir.AluOpType.mult, mybir.AluOpType.max

    groups = [list(range(g * G, (g + 1) * G)) for g in range(NC // G)]

    nc = bacc.Bacc("TRN2", target_bir_lowering=False, debug=False, num_devices=NC)

    def din(name, shape, dt=BF16):
        return nc.dram_tensor(name, shape, dt, kind="ExternalInput").ap()

    x_d = din("x", [128, ET * TOK], F32)
    wq_d = din("wq", [128, ET * ET * 128])
    wk_d = din("wk", [128, ET * ET * 128])
    wv_d = din("wv", [128, ET * E])           # row-major (moving operand)
    wproj_d = din("wproj", [128, ET * ET * 128])
    w1_d = din("w1", [128, FT * ET * 128])
    w2_d = din("w2", [128, ET * FT * 128])
    g1_d = din("g1c", [128, ET], F32)
    be1_d = din("be1c", [128, ET], F32)
    g2_d = din("g2c", [128, ET], F32)
    be2_d = din("be2c", [128, ET], F32)
    bproj_d = din("bprojc", [128, ET], F32)
    b1_d = din("b1c", [128, FT], F32)
    b2_d = din("b2c", [128, ET], F32)
    out_d = nc.dram_tensor("outT", [128, ET * TOK], F32, kind="ExternalOutput").ap()

    ones_col_bf = nc.const_aps.tensor(1.0, (128, 1), BF16)
    zeros_bc = nc.const_aps.tensor(0.0, (128, TOK), F32)

    kvslot = max(ET * E, G * ET * TOK, G * TOKT * H * HD1, FT * TOK)

    with tile.TileContext(nc) as tc:
        with (
            tc.tile_pool(name="dram", bufs=1, space="DRAM") as dram,
            tc.tile_pool(name="resid", bufs=2) as resid,     # x, y, out (f32, shared)
            tc.tile_pool(name="bigkv", bufs=2) as bigkv,     # wv, ktall, vall, f
            tc.tile_pool(name="acts", bufs=4) as acts,       # bf16 activations, shared
            tc.tile_pool(name="small", bufs=1) as small,     # cols, ones
            tc.tile_pool(name="wstr", bufs=3) as wstr,       # streamed weight blocks
            tc.tile_pool(name="wstr2", bufs=2) as wstr2,     # streamed w2 blocks
            tc.tile_pool(name="rows", bufs=1) as rows,       # [1, TOK] scalar rows
            tc.tile_pool(name="rr", bufs=2) as rr,           # per-head recip rows
            tc.tile_pool(name="scr", bufs=1) as scr,         # scratch [128, TOK]
            tc.tile_pool(name="expp", bufs=2) as expp,       # exp tiles
            tc.tile_pool(name="shp", bufs=2) as shp,         # odd-head shift staging
        ):
            # ---- constant / input loads ----
            x_sb = resid.tile([128, ET * TOK], F32, tag="resid")
            nc.sync.dma_start(x_sb[:], x_d)
            cols = small.tile([128, 6 * ET + FT], F32, tag="cols")
            for i, d in enumerate([g1_d, be1_d, g2_d, be2_d, bproj_d, b2_d]):
                nc.sync.dma_start(cols[:, i * ET:(i + 1) * ET], d)
            nc.sync.dma_start(cols[:, 6 * ET:6 * ET + FT], b1_d)
            g1c = cols[:, 0 * ET:1 * ET]
            be1c = cols[:, 1 * ET:2 * ET]
            g2c = cols[:, 2 * ET:3 * ET]
            be2c = cols[:, 3 * ET:4 * ET]
            bprojc = cols[:, 4 * ET:5 * ET]
            b2c = cols[:, 5 * ET:6 * ET]
            b1c = cols[:, 6 * ET:6 * ET + FT]
            ones_full = small.tile([128, 128], BF16, tag="ones")
            nc.vector.memset(ones_full[:], 1.0)
            wv_sb = bigkv.tile([128, kvslot], BF16, tag="kv")
            nc.sync.dma_start(wv_sb[:, 0:ET * E], wv_d)

            def layernorm(src_f32, dst_bf, g_cols, b_cols):
                """src [128, ET*TOK] f32 -> dst [128, ET*TOK] bf16 (feature-major)."""
                x_bf = acts.tile([128, ET * TOK], BF16, tag="act8")
                sq = acts.tile([128, ET * TOK], BF16, tag="act8")
                with tc.tile_pool(name="lnps", bufs=1, space="PSUM") as lnps:
                    for k in range(ET):
                        sl = slice(k * TOK, (k + 1) * TOK)
                        nc.vector.tensor_copy(x_bf[:, sl], src_f32[:, sl])
                        nc.vector.tensor_mul(sq[:, sl], x_bf[:, sl], x_bf[:, sl])
                    st_s = lnps.tile([1, TOK], F32, tag="st_s")
                    st_q = lnps.tile([1, TOK], F32, tag="st_q")
                    for k in range(ET):
                        sl = slice(k * TOK, (k + 1) * TOK)
                        nc.tensor.matmul(st_s[:], ones_col_bf, x_bf[:, sl],
                                         start=(k == 0), stop=(k == ET - 1))
                    for k in range(ET):
                        sl = slice(k * TOK, (k + 1) * TOK)
                        nc.tensor.matmul(st_q[:], ones_col_bf, sq[:, sl],
                                         start=(k == 0), stop=(k == ET - 1))
                    r_mean = rows.tile([1, TOK], F32, tag="rowf")
                    r_m2 = rows.tile([1, TOK], F32, tag="rowf2")
                    r_msq = rows.tile([1, TOK], F32, tag="rowf3")
                    r_var = rows.tile([1, TOK], F32, tag="rowf2b")
                    r_rec = rows.tile([1, TOK], F32, tag="rowf3b")
                    r_rstd = rows.tile([1, TOK], F32, tag="rowf4")
                    r_nmr = rows.tile([1, TOK], F32, tag="rowf5")
                    nc.vector.tensor_scalar_mul(r_mean[:], st_s[:], 1.0 / E)
                    nc.vector.tensor_scalar_mul(r_m2[:], st_q[:], 1.0 / E)
                    nc.vector.tensor_mul(r_msq[:], r_mean[:], r_mean[:])
                    nc.vector.tensor_sub(r_var[:], r_m2[:], r_msq[:])
                    nc.vector.tensor_scalar_add(r_var[:], r_var[:], eps)
                    nc.vector.reciprocal(r_rec[:], r_var[:])
                    nc.scalar.sqrt(r_rstd[:], r_rec[:])
                    nc.vector.scalar_tensor_tensor(
                        out=r_nmr[:], in0=r_mean[:], scalar=-1.0, in1=r_rstd[:],
                        op0=mult, op1=mult)
                    r_rstd_bf = rows.tile([1, TOK], BF16, tag="rowbf")
                    r_nmr_bf = rows.tile([1, TOK], BF16, tag="rowbf2")
                    nc.vector.tensor_copy(r_rstd_bf[:], r_rstd[:])
                    nc.vector.tensor_copy(r_nmr_bf[:], r_nmr[:])
                    with tc.tile_pool(name="lnbc", bufs=1, space="PSUM") as lnbc:
                        ps_rstd = lnbc.tile([128, TOK], F32, tag="bc_rstd")
                        ps_nmr = lnbc.tile([128, TOK], F32, tag="bc_nmr")
                        nc.tensor.matmul(ps_rstd[:], ones_full[0:1, :], r_rstd_bf[:],
                                         start=True, stop=True)
                        nc.tensor.matmul(ps_nmr[:], ones_full[0:1, :], r_nmr_bf[:],
                                         start=True, stop=True)
                        for k in range(ET):
                            sl = slice(k * TOK, (k + 1) * TOK)
                            t1 = scr.tile([128, TOK], F32, tag="ln_t1")
                            t2 = scr.tile([128, TOK], F32, tag="ln_t2")
                            nc.vector.tensor_mul(t1[:], src_f32[:, sl], ps_rstd[:])
                            nc.vector.tensor_add(t2[:], t1[:], ps_nmr[:])
                            nc.vector.scalar_tensor_tensor(
                                out=dst_bf[:, sl], in0=t2[:],
                                scalar=g_cols[:, k:k + 1],
                                in1=b_cols[:, k:k + 1].to_broadcast((128, TOK)),
                                op0=mult, op1=add)

            # ================= LN1 =================
            h_bf = acts.tile([128, ET * TOK], BF16, tag="act8")
            layernorm(x_sb, h_bf, g1c, be1c)

            # ================= K^T, V (+ AllGather), Q =================
            kbounce = dram.tile([128, ET * TOK], BF16, tag="kb")
            kall = dram.tile([G * 128, ET * TOK], BF16, tag="ka")
            vbounce = dram.tile([128, TOKT * H * HD1], BF16, tag="vb")
            vall = dram.tile([G * 128, TOKT * H * HD1], BF16, tag="va")

            with tc.tile_pool(name="qkvps", bufs=2, space="PSUM") as qkvps:
                ktloc = acts.tile([128, ET * TOK], BF16, tag="act8")
                for m in range(ET):
                    wblk = wstr.tile([128, ET * 128], BF16, tag="wa")
                    nc.sync.dma_start(wblk[:], wk_d[:, m * ET * 128:(m + 1) * ET * 128])
                    ps = qkvps.tile([128, TOK], F32, tag="mm")
                    for k in range(ET):
                        nc.tensor.matmul(ps[:], wblk[:, k * 128:(k + 1) * 128],
                                         h_bf[:, k * TOK:(k + 1) * TOK],
                                         start=(k == 0), stop=(k == ET - 1))
                    nc.vector.tensor_copy(ktloc[:, m * TOK:(m + 1) * TOK], ps[:])
                nc.sync.dma_start(kbounce[:], ktloc[:])
                nc.gpsimd.collective_compute(
                    "AllGather", mybir.AluOpType.bypass, replica_groups=groups,
                    ins=[kbounce.opt()], outs=[kall.opt()])

                vloc = acts.tile([128, TOKT * H * HD1], BF16, tag="act8")
                vloc4 = vloc[:].rearrange("p (t h d) -> p t h d", t=TOKT, h=H, d=HD1)
                nc.vector.memset(vloc4[:, :, :, DH:DH + 1], 1.0)
                for tt in range(TOKT):
                    for nn in range(NV):
                        w = min(512, H * DH - nn * 512)
                        ps = qkvps.tile([128, max(TOK, 512)], F32, tag="mmv")
                        for k in range(ET):
                            nc.tensor.matmul(
                                ps[:, 0:w],
                                h_bf[:, k * TOK + tt * 128: k * TOK + tt * 128 + 128],
                                wv_sb[:, k * E + nn * 512: k * E + nn * 512 + w],
                                start=(k == 0), stop=(k == ET - 1))
                        nhd = w // DH
                        src = ps[:, 0:w].rearrange("p (h d) -> p h d", h=nhd, d=DH)
                        dst = vloc4[:, tt:tt + 1, nn * HPN:nn * HPN + nhd, 0:DH]
                        nc.vector.tensor_copy(dst.opt(), src)
                nc.sync.dma_start(vbounce[:], vloc[:])
                nc.gpsimd.collective_compute(
                    "AllGather", mybir.AluOpType.bypass, replica_groups=groups,
                    ins=[vbounce.opt()], outs=[vall.opt()])

                q_sb = acts.tile([128, ET * TOK], BF16, tag="act8")
                for m in range(ET):
                    wblk = wstr.tile([128, ET * 128], BF16, tag="wa")
                    nc.sync.dma_start(wblk[:], wq_d[:, m * ET * 128:(m + 1) * ET * 128])
                    ps = qkvps.tile([128, TOK], F32, tag="mm")
                    for k in range(ET):
                        nc.tensor.matmul(ps[:], wblk[:, k * 128:(k + 1) * 128],
                                         h_bf[:, k * TOK:(k + 1) * TOK],
                                         start=(k == 0), stop=(k == ET - 1))
                    nc.vector.tensor_copy(q_sb[:, m * TOK:(m + 1) * TOK], ps[:])

            # ================= attention =================
            kt_all = bigkv.tile([128, kvslot], BF16, tag="kv")
            v_all = bigkv.tile([128, kvslot], BF16, tag="kv")
            for cc in range(G):
                nc.sync.dma_start(
                    kt_all[:, cc * ET * TOK:(cc + 1) * ET * TOK],
                    kall[cc * 128:(cc + 1) * 128, :])
                nc.sync.dma_start(
                    v_all[:, cc * TOKT * H * HD1:(cc + 1) * TOKT * H * HD1],
                    vall[cc * 128:(cc + 1) * 128, :])

            attn_sb = acts.tile([128, ET * TOK], BF16, tag="act8")

            def kt_slice(h, kt):
                cc, l = divmod(kt, TOKT)
                base = (h % 2) * 64
                off = (cc * ET + h // 2) * TOK + l * 128
                return kt_all[base:base + 64, off:off + 128]

            def q_slice(h):
                base = (h % 2) * 64
                return q_sb[base:base + 64, (h // 2) * TOK:(h // 2 + 1) * TOK]

            def v_slice(h, kt):
                off = kt * H * HD1 + h * HD1
                return v_all[:, off:off + HD1]

            with (
                tc.tile_pool(name="sps", bufs=1, space="PSUM") as sps,
                tc.tile_pool(name="avps", bufs=1, space="PSUM") as avps,
                tc.tile_pool(name="bcps", bufs=2, space="PSUM") as bcps,
            ):
                for hp in range(H // 2):
                    h0, h1 = 2 * hp, 2 * hp + 1
                    av0 = avps.tile([HD1, TOK], F32, tag="av0")
                    av1 = avps.tile([HD1, TOK], F32, tag="av1")
                    for g2 in range(KT // 2):
                        s0 = sps.tile([128, 2 * TOK], F32, tag="s0")
                        s1 = sps.tile([128, 2 * TOK], F32, tag="s1")
                        for j in range(2):
                            kt = 2 * g2 + j
                            nc.tensor.matmul(s0[:, j * TOK:(j + 1) * TOK],
                                             kt_slice(h0, kt), q_slice(h0),
                                             start=True, stop=True)
                            nc.tensor.matmul(s1[:, j * TOK:(j + 1) * TOK],
                                             kt_slice(h1, kt), q_slice(h1),
                                             start=True, stop=True)
                        e0 = expp.tile([128, 2 * TOK], BF16, tag="e0")
                        e1 = expp.tile([128, 2 * TOK], BF16, tag="e1")
                        nc.scalar.activation(e0[:], s0[:],
                                             mybir.ActivationFunctionType.Exp,
                                             scale=sm_scale)
                        nc.scalar.activation(e1[:], s1[:],
                                             mybir.ActivationFunctionType.Exp,
                                             scale=sm_scale)
                        for j in range(2):
                            kt = 2 * g2 + j
                            nc.tensor.matmul(av0[:], v_slice(h0, kt),
                                             e0[:, j * TOK:(j + 1) * TOK],
                                             start=(kt == 0), stop=(kt == KT - 1))
                            nc.tensor.matmul(av1[:], v_slice(h1, kt),
                                             e1[:, j * TOK:(j + 1) * TOK],
                                             start=(kt == 0), stop=(kt == KT - 1))
                    # normalize by the gathered denominator (row DH of av psum)
                    for h, av in ((h0, av0), (h1, av1)):
                        rrec = rr.tile([128, TOK], F32, tag="rrec")
                        rrecb = rr.tile([128, TOK], BF16, tag="rrecb")
                        nc.vector.reciprocal(rrec[DH:DH + 1, :], av[DH:DH + 1, :])
                        nc.vector.tensor_copy(rrecb[DH:DH + 1, :], rrec[DH:DH + 1, :])
                        psr = bcps.tile([128, TOK], F32, tag="psr")
                        nc.tensor.matmul(psr[:], ones_full[DH:DH + 1, :],
                                         rrecb[DH:DH + 1, :], start=True, stop=True)
                        # DVE cannot read two PSUM operands; stage av in SBUF
                        avsb = shp.tile([64, TOK], F32, tag="avsb")
                        nc.vector.tensor_copy(avsb[:], av[0:DH, :])
                        if h % 2 == 0:
                            nc.vector.tensor_mul(
                                attn_sb[0:DH, hp * TOK:(hp + 1) * TOK],
                                avsb[:], psr[0:DH, :])
                        else:
                            tmp = shp.tile([64, TOK], BF16, tag="shift")
                            nc.vector.tensor_mul(tmp[:], avsb[:], psr[0:DH, :])
                            nc.sync.dma_start(
                                attn_sb[64:128, hp * TOK:(hp + 1) * TOK], tmp[:])

            # ================= proj + residual =================
            y_sb = resid.tile([128, ET * TOK], F32, tag="resid")
            with tc.tile_pool(name="prps", bufs=2, space="PSUM") as prps:
                for m in range(ET):
                    wblk = wstr.tile([128, ET * 128], BF16, tag="wa")
                    nc.sync.dma_start(wblk[:],
                                      wproj_d[:, m * ET * 128:(m + 1) * ET * 128])
                    ps = prps.tile([128, TOK], F32, tag="mm")
                    for k in range(ET):
                        nc.tensor.matmul(ps[:], wblk[:, k * 128:(k + 1) * 128],
                                         attn_sb[:, k * TOK:(k + 1) * TOK],
                                         start=(k == 0), stop=(k == ET - 1))
                    nc.vector.scalar_tensor_tensor(
                        out=y_sb[:, m * TOK:(m + 1) * TOK], in0=ps[:],
                        scalar=bprojc[:, m:m + 1],
                        in1=x_sb[:, m * TOK:(m + 1) * TOK], op0=add, op1=add)

            # ================= LN2 =================
            h2_bf = acts.tile([128, ET * TOK], BF16, tag="act8")
            layernorm(y_sb, h2_bf, g2c, be2c)

            # ================= FFN =================
            f_bf = bigkv.tile([128, kvslot], BF16, tag="kv")
            with tc.tile_pool(name="f1ps", bufs=3, space="PSUM") as f1ps:
                for m in range(FT):
                    wblk = wstr.tile([128, ET * 128], BF16, tag="wa")
                    nc.sync.dma_start(wblk[:],
                                      w1_d[:, m * ET * 128:(m + 1) * ET * 128])
                    ps = f1ps.tile([128, TOK], F32, tag="mm")
                    for k in range(ET):
                        nc.tensor.matmul(ps[:], wblk[:, k * 128:(k + 1) * 128],
                                         h2_bf[:, k * TOK:(k + 1) * TOK],
                                         start=(k == 0), stop=(k == ET - 1))
                    nc.vector.scalar_tensor_tensor(
                        out=f_bf[:, m * TOK:(m + 1) * TOK], in0=ps[:],
                        scalar=b1c[:, m:m + 1], in1=zeros_bc, op0=add, op1=mx)

            out_sb = resid.tile([128, ET * TOK], F32, tag="resid")
            with tc.tile_pool(name="f2ps", bufs=3, space="PSUM") as f2ps:
                for m in range(ET):
                    wblk2 = wstr2.tile([128, FT * 128], BF16, tag="wb")
                    nc.sync.dma_start(wblk2[:],
                                      w2_d[:, m * FT * 128:(m + 1) * FT * 128])
                    ps = f2ps.tile([128, TOK], F32, tag="mm")
                    for k in range(FT):
                        nc.tensor.matmul(ps[:], wblk2[:, k * 128:(k + 1) * 128],
                                         f_bf[:, k * TOK:(k + 1) * TOK],
                                         start=(k == 0), stop=(k == FT - 1))
                    nc.vector.scalar_tensor_tensor(
                        out=out_sb[:, m * TOK:(m + 1) * TOK], in0=ps[:],
                        scalar=b2c[:, m:m + 1],
                        in1=y_sb[:, m * TOK:(m + 1) * TOK], op0=add, op1=add)
                    nc.sync.dma_start(out_d[:, m * TOK:(m + 1) * TOK],
                                      out_sb[:, m * TOK:(m + 1) * TOK])

    nc.compile()
    return nc


# ---------------- host-side packing ----------------

def _colblk(w2d, kt, mt):
    """[kt*128, mt*128] -> [128, mt, kt, 128] flattened (weight as lhsT blocks)."""
    return np.ascontiguousarray(
        w2d.reshape(kt, 128, mt, 128).transpose(1, 2, 0, 3).reshape(128, mt * kt * 128))


def _rowmaj(w2d, kt):
    """[kt*128, N] -> [128, kt, N] flattened (weight as moving operand)."""
    n = w2d.shape[1]
    return np.ascontiguousarray(
        w2d.reshape(kt, 128, n).transpose(1, 0, 2).reshape(128, kt * n))


def _fm(chunk_te, et, tok):
    """[TOK, E] token-major -> [128, ET*TOK] feature-major SBUF layout."""
    return np.ascontiguousarray(
        chunk_te.T.reshape(et, 128, tok).transpose(1, 0, 2).reshape(128, et * tok))


def _cols(v, t):
    return np.ascontiguousarray(v.reshape(t, 128).T)


def make_in_maps(dims, x, Wq, Wk, Wv, Wproj, bproj, W1, b1, W2, b2,
                 g1, beta1, g2, beta2):
    E, H, DH, TOK, G, NC, FF = (
        dims["E"], dims["H"], dims["DH"], dims["TOK"], dims["G"], dims["NC"], dims["FF"]
    )
    ET, FT = E // 128, FF // 128
    bf = ml_dtypes.bfloat16
    wq2 = Wq.transpose(1, 0, 2).reshape(E, H * DH)
    wk2 = Wk.transpose(1, 0, 2).reshape(E, H * DH)
    wv2 = Wv.transpose(1, 0, 2).reshape(E, H * DH)
    shared = {
        "wq": _colblk(wq2, ET, (H * DH) // 128).astype(bf),
        "wk": _colblk(wk2, ET, (H * DH) // 128).astype(bf),
        "wv": _rowmaj(wv2, ET).astype(bf),
        "wproj": _colblk(Wproj, (H * DH) // 128, ET).astype(bf),
        "w1": _colblk(W1, ET, FT).astype(bf),
        "w2": _colblk(W2, FT, ET).astype(bf),
        "g1c": _cols(g1, ET).astype(np.float32),
        "be1c": _cols(beta1, ET).astype(np.float32),
        "g2c": _cols(g2, ET).astype(np.float32),
        "be2c": _cols(beta2, ET).astype(np.float32),
        "bprojc": _cols(bproj, ET).astype(np.float32),
        "b1c": _cols(b1, FT).astype(np.float32),
        "b2c": _cols(b2, ET).astype(np.float32),
    }
    xf = x.reshape(-1, E)  # [B*S, E]
    in_maps = []
    for r in range(NC):
        xc = xf[r * TOK:(r + 1) * TOK, :]
        m = dict(shared)
        m["x"] = _fm(xc.astype(np.float32), ET, TOK)
        in_maps.append(m)
    return in_maps


def assemble_out(dims, results):
    E, TOK, NC = dims["E"], dims["TOK"], dims["NC"]
    ET = E // 128
    outs = []
    for r in range(NC):
        o = results[r]["outT"]  # [128, ET*TOK]
        outs.append(o.reshape(128, ET, TOK).transpose(1, 0, 2).reshape(E, TOK).T)
    return np.concatenate(outs, axis=0)  # [B*S, E]


_NC_CACHE = {}


def kernel(x, Wq, Wk, Wv, Wproj, bproj, W1, b1, W2, b2, g1, beta1, g2, beta2,
           **extra):
    dims = FULL_DIMS
    arrs = dict(x=np.asarray(x, np.float32))
    for k, v in dict(Wq=Wq, Wk=Wk, Wv=Wv, Wproj=Wproj, bproj=bproj, W1=W1,
                     b1=b1, W2=W2, b2=b2, g1=g1, beta1=beta1, g2=g2,
                     beta2=beta2).items():
        arrs[k] = np.asarray(v, np.float32)
    in_maps = make_in_maps(dims, **arrs)
    key = "full"
    if key not in _NC_CACHE:
        _NC_CACHE[key] = build_nc(dims)
    nc = _NC_CACHE[key]
    res = run_bass_kernel_spmd(nc, in_maps, core_ids=list(range(dims["NC"])))
    flat = assemble_out(dims, res.results)
    B = x.shape[0]
    return flat.reshape(B, -1, dims["E"]).astype(np.float32)
